# revision 22
# baseline (speedup 1.0000x reference)
"""Multi-head attention TRN2 kernel: 8 cores = 4 batch x 2 head-groups.

Per core (b = core//2, g = core%2): computes the attention block for batch
sample b restricted to heads [8g, 8g+8), producing the (transposed) partial
output projection. Host sums the two head-group partials per batch + bias.

Layouts (per core, host-prepped):
  xq/xk/xv : x^T        [1024 d, 2048 t] f32  (fp32r matmul operands)
  wq/wk/wv : W_g^T      [1024 d, 512 j]  f32
  wo       : Wo_g^T     [512 c, 1024 j]  bf16
  out      : OUT^T partial [1024 j, 2048 t] f32
"""

import numpy as np
import ml_dtypes

D = 1024          # d_model
L = 2048          # sequence length
B = 4             # batch
HG = 512          # head-group width (8 heads x 64)
NCORES = 8
EXP_BIAS = -45.0  # softmax shift: exp(S-45); cancels in normalization

NT = 4            # token chunks of 512
TC = L // NT      # 512
NDB = D // 128    # 8 d-model blocks
NP = 4            # head pairs per group
NKB = L // 128    # 16 key blocks

_COMPILED = None
LAST_RESULT = None


def _build():
    import concourse.bacc as bacc
    import concourse.mybir as mybir
    import concourse.tile as tile

    f32 = mybir.dt.float32
    f32r = mybir.dt.float32r
    bf16 = mybir.dt.bfloat16
    EXP = mybir.ActivationFunctionType.Exp
    ADD = mybir.AluOpType.add
    MUL = mybir.AluOpType.mult

    nc = bacc.Bacc()

    xq = nc.declare_dram_parameter("xq", [D, L], f32r, isOutput=False)
    xk = nc.declare_dram_parameter("xk", [D, L], f32r, isOutput=False)
    xv = nc.declare_dram_parameter("xv", [D, L], bf16, isOutput=False)
    wq = nc.declare_dram_parameter("wq", [D, HG], f32r, isOutput=False)
    wk = nc.declare_dram_parameter("wk", [D, HG], f32r, isOutput=False)
    wv = nc.declare_dram_parameter("wv", [D, HG], bf16, isOutput=False)
    wo = nc.declare_dram_parameter("wo", [HG, D], bf16, isOutput=False)
    bq = nc.declare_dram_parameter("bq", [HG], f32, isOutput=False)
    bv = nc.declare_dram_parameter("bv", [HG], f32, isOutput=False)
    out = nc.declare_dram_parameter("out", [D, L], f32, isOutput=True)

    # DRAM scratch for Q^T (f32r), streamed back during attention
    qt_dram = nc.dram_tensor("qt_dram", [HG, L], f32r)
    qt_v = qt_dram.rearrange("(jb p) (n t) -> jb p n t", p=128, t=TC)

    with tile.TileContext(nc) as tc:
        with tc.tile_pool(name="res", bufs=1) as res, tc.tile_pool(
            name="psum", bufs=1, space="PSUM"
        ) as psum:
            # ---- resident tiles (DMAs emitted lazily near first use) ----
            wo_sb = res.tile([128, NP, D], bf16)
            bq_sb = res.tile([128, NP], f32)
            nc.sync.dma_start(out=bq_sb[:], in_=bq.rearrange("(jb p) -> p jb", p=128))
            bv_row = res.tile([1, HG], f32)
            nc.sync.dma_start(out=bv_row[:], in_=bv.rearrange("(o j) -> o j", o=1))
            bv_bc = res.tile([128, HG], f32)
            nc.gpsimd.partition_broadcast(bv_bc[:], bv_row[:], channels=128)
            bias_exp = res.tile([128, 1], f32)
            nc.vector.memset(bias_exp[:], EXP_BIAS)

            # V in AV-stationary layout: per (kb, pair): [Vh_even, 1, Vh_odd, 1]
            v_sb = res.tile([128, NKB, NP, 130], bf16)
            nc.vector.memset(v_sb[:, :, :, 64:65], 1.0)
            nc.vector.memset(v_sb[:, :, :, 129:130], 1.0)

            ct_sb = res.tile([128, NP, NT, TC], bf16)  # full C^T, bf16
            kt_sb = res.tile([128, NP, L], f32r)       # resident K^T

            # ---- projection phase (scoped pools, freed before attention) ----
            with tc.tile_pool(name="pw", bufs=1) as pw, tc.tile_pool(
                name="px", bufs=1
            ) as px:
                wk_sb = pw.tile([128, NDB, HG], f32r)
                wq_sb = pw.tile([128, NDB, HG], f32r)
                wv_sb = pw.tile([128, NDB, HG], bf16)
                nc.sync.dma_start(
                    out=wk_sb[:], in_=wk.rearrange("(db p) j -> p db j", p=128)
                )

                def load_x_chunk(src, t, dt=f32r):
                    xt = px.tile([128, NDB, TC], dt, name="xt", tag="xt", bufs=2)
                    nc.sync.dma_start(
                        out=xt[:],
                        in_=src.rearrange("(db p) (n t) -> p db n t", p=128, t=TC)[
                            :, :, t
                        ],
                    )
                    return xt

                # K^T projection -> kt_dram
                for t in range(NT):
                    xk_t = load_x_chunk(xk, t)
                    for jb in range(NP):
                        ps = psum.tile([128, TC], f32, name="acc", tag="accu", bufs=2)
                        for db in range(NDB):
                            nc.tensor.matmul(
                                ps[:],
                                wk_sb[:, db, jb * 128 : (jb + 1) * 128],
                                xk_t[:, db, :],
                                start=(db == 0),
                                stop=(db == NDB - 1),
                            )
                        nc.vector.tensor_copy(
                            kt_sb[:, jb, t * TC : (t + 1) * TC], ps[:]
                        )

                # V projection (natural layout, +bias) -> v_sb
                nc.sync.dma_start(
                    out=wv_sb[:], in_=wv.rearrange("(db p) j -> p db j", p=128)
                )
                for t in range(NT):
                    xv_t = load_x_chunk(xv, t, dt=bf16)
                    for tb in range(4):
                        kb = t * 4 + tb
                        ps = psum.tile([128, HG], f32, name="acc", tag="accu", bufs=2)
                        for db in range(NDB):
                            nc.tensor.matmul(
                                ps[:],
                                xv_t[:, db, tb * 128 : (tb + 1) * 128],
                                wv_sb[:, db, :],
                                start=(db == 0),
                                stop=(db == NDB - 1),
                            )
                        for p in range(NP):
                            nc.vector.tensor_tensor(
                                out=v_sb[:, kb, p, 0:64],
                                in0=ps[:, p * 128 : p * 128 + 64],
                                in1=bv_bc[:, p * 128 : p * 128 + 64],
                                op=ADD,
                            )
                            nc.vector.tensor_tensor(
                                out=v_sb[:, kb, p, 65:129],
                                in0=ps[:, p * 128 + 64 : p * 128 + 128],
                                in1=bv_bc[:, p * 128 + 64 : p * 128 + 128],
                                op=ADD,
                            )

                # Q^T projection (+bias per partition) -> qt_dram
                nc.sync.dma_start(
                    out=wq_sb[:], in_=wq.rearrange("(db p) j -> p db j", p=128)
                )
                for t in range(NT):
                    xq_t = load_x_chunk(xq, t)
                    for jb in range(NP):
                        ps = psum.tile([128, TC], f32, name="acc", tag="accu", bufs=2)
                        for db in range(NDB):
                            nc.tensor.matmul(
                                ps[:],
                                wq_sb[:, db, jb * 128 : (jb + 1) * 128],
                                xq_t[:, db, :],
                                start=(db == 0),
                                stop=(db == NDB - 1),
                            )
                        po = px.tile([128, TC], f32r, name="po", tag="po", bufs=3)
                        nc.vector.tensor_scalar_add(po[:], ps[:], bq_sb[:, jb : jb + 1])
                        nc.sync.dma_start(out=qt_v[jb, :, t], in_=po[:])

            # ---- attention (chunk-outer, pair-inner) + inline out-proj ----
            with tc.tile_pool(name="pa", bufs=1) as pa:
                nc.sync.dma_start(
                    out=wo_sb[:], in_=wo.rearrange("(cb p) j -> p cb j", p=128)
                )
                for q in range(NT):
                    for p in range(NP):
                        qt_c = pa.tile([128, TC], f32r, name="qt_c", tag="qt_c", bufs=3)
                        nc.sync.dma_start(out=qt_c[:], in_=qt_v[p, :, q])
                        # P^T for both heads: [kb][head e/o][q]
                        pt = pa.tile([128, NKB, 2, TC], bf16, name="pt", tag="pt", bufs=2)
                        for kb in range(NKB):
                            ps_s = psum.tile(
                                [128, 2, TC], f32, name="ps_s", tag="ps_s", bufs=2
                            )
                            nc.tensor.matmul(
                                ps_s[:, 0, :],
                                kt_sb[0:64, p, kb * 128 : (kb + 1) * 128],
                                qt_c[0:64, :],
                                start=True,
                                stop=True,
                            )
                            nc.tensor.matmul(
                                ps_s[:, 1, :],
                                kt_sb[64:128, p, kb * 128 : (kb + 1) * 128],
                                qt_c[64:128, :],
                                start=True,
                                stop=True,
                            )
                            nc.scalar.activation(
                                pt[:, kb, :, :], ps_s[:], EXP,
                                bias=bias_exp[:], scale=1.0,
                            )
                        # AV: U^T + rowsum via ones column (M=65)
                        ps_u = psum.tile([128, 2, TC], f32, name="ps_u", tag="accu", bufs=2)
                        for kb in range(NKB):
                            nc.tensor.matmul(
                                ps_u[0:65, 0, :],
                                v_sb[:, kb, p, 0:65],
                                pt[:, kb, 0, :],
                                start=(kb == 0),
                                stop=(kb == NKB - 1),
                            )
                            nc.tensor.matmul(
                                ps_u[0:65, 1, :],
                                v_sb[:, kb, p, 65:130],
                                pt[:, kb, 1, :],
                                start=(kb == 0),
                                stop=(kb == NKB - 1),
                            )
                        # normalize: C^T = U^T * (1/r).
                        rr_e = pa.tile([1, TC], f32, name="rr_e", tag="rr_e", bufs=2)
                        rr_o = pa.tile([1, TC], f32, name="rr_o", tag="rr_o", bufs=2)
                        nc.vector.tensor_copy(rr_e[:], ps_u[64:65, 0, :])
                        nc.vector.tensor_copy(rr_o[:], ps_u[64:65, 1, :])
                        r128 = pa.tile([128, 8], f32, name="r128", tag="r128", bufs=2)
                        nc.sync.dma_start(out=r128[:, 0:4], in_=rr_e[:])
                        nc.sync.dma_start(out=r128[:, 4:8], in_=rr_o[:])
                        nc.vector.reciprocal(r128[:], r128[:])
                        rv_e = pa.tile([1, TC], f32, name="rv_e", tag="rv_e", bufs=2)
                        rv_o = pa.tile([1, TC], f32, name="rv_o", tag="rv_o", bufs=2)
                        nc.sync.dma_start(out=rv_e[:], in_=r128[:, 0:4])
                        nc.sync.dma_start(out=rv_o[:], in_=r128[:, 4:8])
                        rb_e = pa.tile([64, TC], f32, name="rb_e", tag="rb_e", bufs=2)
                        rb_o = pa.tile([64, TC], f32, name="rb_o", tag="rb_o", bufs=2)
                        nc.gpsimd.partition_broadcast(rb_e[:], rv_e[:], channels=64)
                        nc.gpsimd.partition_broadcast(rb_o[:], rv_o[:], channels=64)
                        nc.vector.tensor_tensor(
                            out=ct_sb[0:64, p, q, :],
                            in0=ps_u[0:64, 0, :],
                            in1=rb_e[:],
                            op=MUL,
                        )
                        # odd head: compute at partitions 0-63, DMA-shift to 64-127
                        ct_o = pa.tile([64, TC], bf16, name="ct_o", tag="ct_o", bufs=2)
                        nc.vector.tensor_tensor(
                            out=ct_o[:], in0=ps_u[0:64, 1, :], in1=rb_o[:], op=MUL
                        )
                        nc.sync.dma_start(out=ct_sb[64:128, p, q, :], in_=ct_o[:])

                    # ---- output projection, delayed one chunk to overlap ----
                    for oq in ([q - 1] if q > 0 else []) + ([q] if q == NT - 1 else []):
                        for ob in range(NDB):
                            ps = psum.tile([128, TC], f32, name="acc", tag="accu", bufs=2)
                            for p in range(NP):
                                nc.tensor.matmul(
                                    ps[:],
                                    wo_sb[:, p, ob * 128 : (ob + 1) * 128],
                                    ct_sb[:, p, oq, :],
                                    start=(p == 0),
                                    stop=(p == NP - 1),
                                )
                            o_sb = pa.tile([128, TC], f32, name="o_sb", tag="o_sb", bufs=3)
                            nc.vector.tensor_copy(o_sb[:], ps[:])
                            nc.sync.dma_start(
                                out=out.rearrange(
                                    "(ob p) (n t) -> ob p n t", p=128, t=TC
                                )[ob, :, oq],
                                in_=o_sb[:],
                            )

    nc.compile()
    return nc


def _get_compiled():
    global _COMPILED
    if _COMPILED is None:
        _COMPILED = _build()
    return _COMPILED


def kernel(q, k, v, Wq, bq, Wk, bk, Wv, bv, Wo, bo):
    global LAST_RESULT
    from concourse.bass_utils import run_bass_kernel_spmd

    nc = _get_compiled()

    q = np.asarray(q, dtype=np.float32)
    k = np.asarray(k, dtype=np.float32)
    v = np.asarray(v, dtype=np.float32)
    Wq = np.asarray(Wq, dtype=np.float32)
    Wk = np.asarray(Wk, dtype=np.float32)
    Wv = np.asarray(Wv, dtype=np.float32)
    Wo = np.asarray(Wo, dtype=np.float32)
    bq = np.asarray(bq, dtype=np.float32)
    bv = np.asarray(bv, dtype=np.float32)
    bo = np.asarray(bo, dtype=np.float32)

    xT = {}
    for b in range(B):
        xT[("q", b)] = np.ascontiguousarray(q[b].T)
        xT[("k", b)] = np.ascontiguousarray(k[b].T)
        xT[("v", b)] = np.ascontiguousarray(v[b].T).astype(ml_dtypes.bfloat16)

    wqT = [np.ascontiguousarray(Wq[g * HG : (g + 1) * HG, :].T) for g in range(2)]
    wkT = [np.ascontiguousarray(Wk[g * HG : (g + 1) * HG, :].T) for g in range(2)]
    wvT = [
        np.ascontiguousarray(Wv[g * HG : (g + 1) * HG, :].T).astype(ml_dtypes.bfloat16)
        for g in range(2)
    ]
    woT = [
        np.ascontiguousarray(Wo[:, g * HG : (g + 1) * HG].T).astype(ml_dtypes.bfloat16)
        for g in range(2)
    ]
    bqg = [np.ascontiguousarray(bq[g * HG : (g + 1) * HG]) for g in range(2)]
    bvg = [np.ascontiguousarray(bv[g * HG : (g + 1) * HG]) for g in range(2)]

    in_maps = []
    for core in range(NCORES):
        b, g = core // 2, core % 2
        in_maps.append(
            {
                "xq": xT[("q", b)],
                "xk": xT[("k", b)],
                "xv": xT[("v", b)],
                "wq": wqT[g],
                "wk": wkT[g],
                "wv": wvT[g],
                "wo": woT[g],
                "bq": bqg[g],
                "bv": bvg[g],
            }
        )

    res = run_bass_kernel_spmd(nc, in_maps, core_ids=list(range(NCORES)))
    LAST_RESULT = res

    outp = np.empty((B, L, D), dtype=np.float32)
    for b in range(B):
        acc = res.results[2 * b]["out"].T + res.results[2 * b + 1]["out"].T
        outp[b] = acc + bo
    return outp


# revision 23
# speedup vs baseline: 1.2016x; 1.2016x over previous
"""Multi-head attention TRN2 kernel: 8 cores = 4 batch x 2 head-groups.

Per core (b = core//2, g = core%2): computes the attention block for batch
sample b restricted to heads [8g, 8g+8), producing the (transposed) partial
output projection. Host sums the two head-group partials per batch + bias.

Layouts (per core, host-prepped):
  xq/xk/xv : x^T        [1024 d, 2048 t] f32  (fp32r matmul operands)
  wq/wk/wv : W_g^T      [1024 d, 512 j]  f32
  wo       : Wo_g^T     [512 c, 1024 j]  bf16
  out      : OUT^T partial [1024 j, 2048 t] f32
"""

import numpy as np
import ml_dtypes

D = 1024          # d_model
L = 2048          # sequence length
B = 4             # batch
HG = 512          # head-group width (8 heads x 64)
NCORES = 8
EXP_BIAS = -45.0  # softmax shift: exp(S-45); cancels in normalization

NT = 4            # token chunks of 512
TC = L // NT      # 512
NDB = D // 128    # 8 d-model blocks
NP = 4            # head pairs per group
NKB = L // 128    # 16 key blocks

_COMPILED = None
LAST_RESULT = None


def _build():
    import concourse.bacc as bacc
    import concourse.mybir as mybir
    import concourse.tile as tile

    f32 = mybir.dt.float32
    f32r = mybir.dt.float32r
    bf16 = mybir.dt.bfloat16
    EXP = mybir.ActivationFunctionType.Exp
    ADD = mybir.AluOpType.add
    MUL = mybir.AluOpType.mult

    nc = bacc.Bacc()

    xq = nc.declare_dram_parameter("xq", [D, L], f32r, isOutput=False)
    xk = nc.declare_dram_parameter("xk", [D, L], f32r, isOutput=False)
    xv = nc.declare_dram_parameter("xv", [D, L], bf16, isOutput=False)
    wq = nc.declare_dram_parameter("wq", [D, HG], f32r, isOutput=False)
    wk = nc.declare_dram_parameter("wk", [D, HG], f32r, isOutput=False)
    wv = nc.declare_dram_parameter("wv", [D, HG], bf16, isOutput=False)
    wo = nc.declare_dram_parameter("wo", [HG, D], bf16, isOutput=False)
    bq = nc.declare_dram_parameter("bq", [HG], f32, isOutput=False)
    bv = nc.declare_dram_parameter("bv", [HG], f32, isOutput=False)
    out = nc.declare_dram_parameter("out", [D, L], f32, isOutput=True)

    # DRAM scratch for Q^T (f32r), streamed back during attention
    qt_dram = nc.dram_tensor("qt_dram", [HG, L], f32r)
    qt_v = qt_dram.rearrange("(jb p) (n t) -> jb p n t", p=128, t=TC)

    with tile.TileContext(nc) as tc:
        with tc.tile_pool(name="res", bufs=1) as res, tc.tile_pool(
            name="psum", bufs=1, space="PSUM"
        ) as psum:
            # ---- resident tiles (DMAs emitted lazily near first use) ----
            wo_sb = res.tile([128, NP, D], bf16)
            bq_sb = res.tile([128, NP], f32)
            nc.sync.dma_start(out=bq_sb[:], in_=bq.rearrange("(jb p) -> p jb", p=128))
            bv_row = res.tile([1, HG], f32)
            nc.sync.dma_start(out=bv_row[:], in_=bv.rearrange("(o j) -> o j", o=1))
            bv_bc = res.tile([128, HG], f32)
            nc.gpsimd.partition_broadcast(bv_bc[:], bv_row[:], channels=128)
            bias_exp = res.tile([128, 1], f32)
            nc.vector.memset(bias_exp[:], EXP_BIAS)

            # V in AV-stationary layout: per (kb, pair): [Vh_even, 1, Vh_odd, 1]
            v_sb = res.tile([128, NKB, NP, 130], bf16)
            nc.vector.memset(v_sb[:, :, :, 64:65], 1.0)
            nc.vector.memset(v_sb[:, :, :, 129:130], 1.0)

            ct_sb = res.tile([128, NP, NT, TC], bf16)  # full C^T, bf16
            kt_sb = res.tile([128, NP, L], f32r)       # resident K^T

            # ---- projection phase (scoped pools, freed before attention) ----
            with tc.tile_pool(name="pw", bufs=1) as pw, tc.tile_pool(
                name="px", bufs=1
            ) as px:
                wk_sb = pw.tile([128, NDB, HG], f32r)
                wq_sb = pw.tile([128, NDB, HG], f32r)
                wv_sb = pw.tile([128, NDB, HG], bf16)
                nc.sync.dma_start(
                    out=wk_sb[:], in_=wk.rearrange("(db p) j -> p db j", p=128)
                )

                def load_x_chunk(src, t, dt=f32r):
                    xt = px.tile([128, NDB, TC], dt, name="xt", tag="xt", bufs=2)
                    nc.sync.dma_start(
                        out=xt[:],
                        in_=src.rearrange("(db p) (n t) -> p db n t", p=128, t=TC)[
                            :, :, t
                        ],
                    )
                    return xt

                # K^T projection -> kt_dram
                for t in range(NT):
                    xk_t = load_x_chunk(xk, t)
                    for jb in range(NP):
                        ps = psum.tile([128, TC], f32, name="acc", tag="accu", bufs=2)
                        for db in range(NDB):
                            nc.tensor.matmul(
                                ps[:],
                                wk_sb[:, db, jb * 128 : (jb + 1) * 128],
                                xk_t[:, db, :],
                                start=(db == 0),
                                stop=(db == NDB - 1),
                            )
                        nc.vector.tensor_copy(
                            kt_sb[:, jb, t * TC : (t + 1) * TC], ps[:]
                        )

                # V projection (natural layout, +bias) -> v_sb
                nc.sync.dma_start(
                    out=wv_sb[:], in_=wv.rearrange("(db p) j -> p db j", p=128)
                )
                for t in range(NT):
                    xv_t = load_x_chunk(xv, t, dt=bf16)
                    for tb in range(4):
                        kb = t * 4 + tb
                        ps = psum.tile([128, HG], f32, name="acc", tag="accu", bufs=2)
                        for db in range(NDB):
                            nc.tensor.matmul(
                                ps[:],
                                xv_t[:, db, tb * 128 : (tb + 1) * 128],
                                wv_sb[:, db, :],
                                start=(db == 0),
                                stop=(db == NDB - 1),
                            )
                        for p in range(NP):
                            nc.vector.tensor_tensor(
                                out=v_sb[:, kb, p, 0:64],
                                in0=ps[:, p * 128 : p * 128 + 64],
                                in1=bv_bc[:, p * 128 : p * 128 + 64],
                                op=ADD,
                            )
                            nc.vector.tensor_tensor(
                                out=v_sb[:, kb, p, 65:129],
                                in0=ps[:, p * 128 + 64 : p * 128 + 128],
                                in1=bv_bc[:, p * 128 + 64 : p * 128 + 128],
                                op=ADD,
                            )

                # Q^T projection (+bias per partition) -> qt_dram
                nc.sync.dma_start(
                    out=wq_sb[:], in_=wq.rearrange("(db p) j -> p db j", p=128)
                )
                for t in range(NT):
                    xq_t = load_x_chunk(xq, t)
                    for jb in range(NP):
                        ps = psum.tile([128, TC], f32, name="acc", tag="accu", bufs=2)
                        for db in range(NDB):
                            nc.tensor.matmul(
                                ps[:],
                                wq_sb[:, db, jb * 128 : (jb + 1) * 128],
                                xq_t[:, db, :],
                                start=(db == 0),
                                stop=(db == NDB - 1),
                            )
                        po = px.tile([128, TC], f32r, name="po", tag="po", bufs=3)
                        nc.vector.tensor_scalar_add(po[:], ps[:], bq_sb[:, jb : jb + 1])
                        nc.sync.dma_start(out=qt_v[jb, :, t], in_=po[:])

            # ---- attention (chunk-outer, pair-inner) + inline out-proj ----
            with tc.tile_pool(name="pa", bufs=1) as pa:
                nc.sync.dma_start(
                    out=wo_sb[:], in_=wo.rearrange("(cb p) j -> p cb j", p=128)
                )
                for p in range(NP):
                    for q in range(NT):
                        qt_c = pa.tile([128, TC], f32r, name="qt_c", tag="qt_c", bufs=3)
                        nc.sync.dma_start(out=qt_c[:], in_=qt_v[p, :, q])
                        # P^T for both heads: [kb][head e/o][q]
                        pt = pa.tile([128, NKB, 2, TC], bf16, name="pt", tag="pt", bufs=2)
                        for kb in range(NKB):
                            ps_s = psum.tile(
                                [128, 2, TC], f32, name="ps_s", tag="ps_s", bufs=2
                            )
                            nc.tensor.matmul(
                                ps_s[:, 0, :],
                                kt_sb[0:64, p, kb * 128 : (kb + 1) * 128],
                                qt_c[0:64, :],
                                start=True,
                                stop=True,
                            )
                            nc.tensor.matmul(
                                ps_s[:, 1, :],
                                kt_sb[64:128, p, kb * 128 : (kb + 1) * 128],
                                qt_c[64:128, :],
                                start=True,
                                stop=True,
                            )
                            nc.scalar.activation(
                                pt[:, kb, :, :], ps_s[:], EXP,
                                bias=bias_exp[:], scale=1.0,
                            )
                        # AV: U^T + rowsum via ones column (M=65)
                        ps_u = psum.tile([128, 2, TC], f32, name="ps_u", tag="accu", bufs=2)
                        for kb in range(NKB):
                            nc.tensor.matmul(
                                ps_u[0:65, 0, :],
                                v_sb[:, kb, p, 0:65],
                                pt[:, kb, 0, :],
                                start=(kb == 0),
                                stop=(kb == NKB - 1),
                            )
                            nc.tensor.matmul(
                                ps_u[0:65, 1, :],
                                v_sb[:, kb, p, 65:130],
                                pt[:, kb, 1, :],
                                start=(kb == 0),
                                stop=(kb == NKB - 1),
                            )
                        # normalize: C^T = U^T * (1/r).
                        rr_e = pa.tile([1, TC], f32, name="rr_e", tag="rr_e", bufs=2)
                        rr_o = pa.tile([1, TC], f32, name="rr_o", tag="rr_o", bufs=2)
                        nc.vector.tensor_copy(rr_e[:], ps_u[64:65, 0, :])
                        nc.vector.tensor_copy(rr_o[:], ps_u[64:65, 1, :])
                        r128 = pa.tile([128, 8], f32, name="r128", tag="r128", bufs=2)
                        nc.sync.dma_start(out=r128[:, 0:4], in_=rr_e[:])
                        nc.sync.dma_start(out=r128[:, 4:8], in_=rr_o[:])
                        nc.vector.reciprocal(r128[:], r128[:])
                        rv_e = pa.tile([1, TC], f32, name="rv_e", tag="rv_e", bufs=2)
                        rv_o = pa.tile([1, TC], f32, name="rv_o", tag="rv_o", bufs=2)
                        nc.sync.dma_start(out=rv_e[:], in_=r128[:, 0:4])
                        nc.sync.dma_start(out=rv_o[:], in_=r128[:, 4:8])
                        rb_e = pa.tile([64, TC], f32, name="rb_e", tag="rb_e", bufs=2)
                        rb_o = pa.tile([64, TC], f32, name="rb_o", tag="rb_o", bufs=2)
                        nc.gpsimd.partition_broadcast(rb_e[:], rv_e[:], channels=64)
                        nc.gpsimd.partition_broadcast(rb_o[:], rv_o[:], channels=64)
                        nc.vector.tensor_tensor(
                            out=ct_sb[0:64, p, q, :],
                            in0=ps_u[0:64, 0, :],
                            in1=rb_e[:],
                            op=MUL,
                        )
                        # odd head: compute at partitions 0-63, DMA-shift to 64-127
                        ct_o = pa.tile([64, TC], bf16, name="ct_o", tag="ct_o", bufs=2)
                        nc.vector.tensor_tensor(
                            out=ct_o[:], in0=ps_u[0:64, 1, :], in1=rb_o[:], op=MUL
                        )
                        nc.sync.dma_start(out=ct_sb[64:128, p, q, :], in_=ct_o[:])

                # ---- output projection (tail; shares ps_s slots) ----
                for oq in range(NT):
                    for ob in range(NDB):
                        ps = psum.tile([128, TC], f32, name="acc", tag="ps_s", bufs=2)
                        for p in range(NP):
                            nc.tensor.matmul(
                                ps[:],
                                wo_sb[:, p, ob * 128 : (ob + 1) * 128],
                                ct_sb[:, p, oq, :],
                                start=(p == 0),
                                stop=(p == NP - 1),
                            )
                        o_sb = pa.tile([128, TC], f32, name="o_sb", tag="o_sb", bufs=3)
                        nc.vector.tensor_copy(o_sb[:], ps[:])
                        nc.sync.dma_start(
                            out=out.rearrange(
                                "(ob p) (n t) -> ob p n t", p=128, t=TC
                            )[ob, :, oq],
                            in_=o_sb[:],
                        )

    nc.compile()
    return nc


def _get_compiled():
    global _COMPILED
    if _COMPILED is None:
        _COMPILED = _build()
    return _COMPILED


def kernel(q, k, v, Wq, bq, Wk, bk, Wv, bv, Wo, bo):
    global LAST_RESULT
    from concourse.bass_utils import run_bass_kernel_spmd

    nc = _get_compiled()

    q = np.asarray(q, dtype=np.float32)
    k = np.asarray(k, dtype=np.float32)
    v = np.asarray(v, dtype=np.float32)
    Wq = np.asarray(Wq, dtype=np.float32)
    Wk = np.asarray(Wk, dtype=np.float32)
    Wv = np.asarray(Wv, dtype=np.float32)
    Wo = np.asarray(Wo, dtype=np.float32)
    bq = np.asarray(bq, dtype=np.float32)
    bv = np.asarray(bv, dtype=np.float32)
    bo = np.asarray(bo, dtype=np.float32)

    xT = {}
    for b in range(B):
        xT[("q", b)] = np.ascontiguousarray(q[b].T)
        xT[("k", b)] = np.ascontiguousarray(k[b].T)
        xT[("v", b)] = np.ascontiguousarray(v[b].T).astype(ml_dtypes.bfloat16)

    wqT = [np.ascontiguousarray(Wq[g * HG : (g + 1) * HG, :].T) for g in range(2)]
    wkT = [np.ascontiguousarray(Wk[g * HG : (g + 1) * HG, :].T) for g in range(2)]
    wvT = [
        np.ascontiguousarray(Wv[g * HG : (g + 1) * HG, :].T).astype(ml_dtypes.bfloat16)
        for g in range(2)
    ]
    woT = [
        np.ascontiguousarray(Wo[:, g * HG : (g + 1) * HG].T).astype(ml_dtypes.bfloat16)
        for g in range(2)
    ]
    bqg = [np.ascontiguousarray(bq[g * HG : (g + 1) * HG]) for g in range(2)]
    bvg = [np.ascontiguousarray(bv[g * HG : (g + 1) * HG]) for g in range(2)]

    in_maps = []
    for core in range(NCORES):
        b, g = core // 2, core % 2
        in_maps.append(
            {
                "xq": xT[("q", b)],
                "xk": xT[("k", b)],
                "xv": xT[("v", b)],
                "wq": wqT[g],
                "wk": wkT[g],
                "wv": wvT[g],
                "wo": woT[g],
                "bq": bqg[g],
                "bv": bvg[g],
            }
        )

    res = run_bass_kernel_spmd(nc, in_maps, core_ids=list(range(NCORES)))
    LAST_RESULT = res

    outp = np.empty((B, L, D), dtype=np.float32)
    for b in range(B):
        acc = res.results[2 * b]["out"].T + res.results[2 * b + 1]["out"].T
        outp[b] = acc + bo
    return outp
